# revision 15
# baseline (speedup 1.0000x reference)
"""CRDLoss Trainium2 kernel (8 NeuronCores, SPMD).

Math: with fs = normalize(f_s @ W_s.T + b_s), ft likewise, s_ij = fs_i.ft_j / T,
L_ij = ln(e^{s_ij} + eps), the reference loss factorizes as

  loss = [ SUM_ij L_ij - SUM_i L_ii + SUM_i s_ii - SUM_c Fss_c.Ft_c
           - N_neg*ln(eps) ] / B

where Fss_c/Ft_c are per-class sums of fs/T and ft (only 100 classes) and
N_neg counts cross-class off-diagonal pairs.  Only SUM_ij L_ij needs the
[B,B] matrix; everything else is tiny and is computed on the host from the
device-produced embeddings.

Device strategy (data-parallel over batch rows, 1024 rows/core):
  - embed local rows of both f_s and f_t: k-chunks arrive via bf16
    DMA-transpose (xbar), PE matmuls accumulate hT in PSUM, bias enters as a
    K=1 matmul, column norms via ACT Square + PE ones-matmul, and
    1/(T*|h|) = exp(-0.5*ln(|h|^2) - ln T) on ACT; scale via a K=1
    outer-product broadcast + DVE multiply.
  - AllGather the local ftT blocks (bf16, 256KB/core) -> ftT_full [128, 8192]
  - sweep the local [1024 x 8192] block of s in [128,2048] PSUM groups:
    PE matmul (bf16, N=512) -> ACT sigmoid(c - s) -> per-row-chunk DVE
    pairwise-product tree to chunk-8 products -> one final ACT ln+accum.
    SUM ln sigmoid(c-s) = SUM (c - L_ij).
"""
import sys

sys.path.insert(0, "/opt/trn_rl_repo")

import numpy as np
import ml_dtypes

import concourse.bass as bass
import concourse.tile as tile
from concourse import mybir
from concourse.bass_utils import run_bass_kernel_spmd

BF16 = ml_dtypes.bfloat16

NCORES = 8
B = 8192            # batch
BLOC = B // NCORES  # rows per core = 1024
S = 2048            # input feature dim (both f_s and f_t)
F = 128             # embed dim
T = 0.07
EPS = 0.97
C = float(np.log(EPS))
KCH = S // 128      # 16 k-chunks
RCH = BLOC // 128   # 8 row chunks per core
GCOLS = 2048        # columns per PSUM group
NGRP = B // GCOLS   # 4 col groups per row chunk


def _fix_multiwaits(nc):
    """walrus codegen accepts only ONE attached sync-wait per instruction.
    Hoist extras onto standalone single-wait NoOps just before, same engine."""
    for fn in nc.m.functions:
        for bb in fn.blocks:
            newl = []
            for inst in bb.instructions:
                si = getattr(inst, "sync_info", None)
                if si is not None and si.on_wait and len(si.on_wait) > 1:
                    extra, keep = si.on_wait[:-1], si.on_wait[-1:]
                    for w in extra:
                        newl.append(mybir.InstNoOp(
                            name=nc.get_next_instruction_name(),
                            engine=inst.engine,
                            sync_info=mybir.SyncInfo(on_wait=[w], on_update=[]),
                            bass_nofuse=True,
                        ))
                    inst.sync_info = mybir.SyncInfo(on_wait=keep, on_update=si.on_update)
                newl.append(inst)
            bb.instructions[:] = newl


def _embed_side(nc, pool, epool, psum_e, consts, x_nat, wT_sb, bias_sb, out_sb):
    """hT[f, i] = sum_k WT[k,f] * x[i,k] + b[f]; out[:, i] = hT[:, i] *
    exp(-0.5*ln(|h_i|^2) + sb).  x_nat: DRAM [BLOC, S] bf16."""
    ones_row, ones_rowf, ones_col, halfneg, sb_ap = consts
    hT = psum_e.tile([128, BLOC], mybir.dt.float32, tag="hT")
    for k in range(KCH):
        xk = pool.tile([128, BLOC], mybir.dt.bfloat16, tag="xchunk")
        nc.sync.dma_start_transpose(out=xk[:], in_=x_nat[0:BLOC, 128 * k:128 * (k + 1)])
        for h in range(BLOC // 512):
            nc.tensor.matmul(hT[:, 512 * h:512 * (h + 1)],
                             wT_sb[:, 128 * k:128 * (k + 1)],
                             xk[:, 512 * h:512 * (h + 1)],
                             start=(k == 0), stop=False)
    for h in range(BLOC // 512):
        nc.tensor.matmul(hT[:, 512 * h:512 * (h + 1)], bias_sb[:],
                         ones_row[:, 0:512], start=False, stop=True)
    # column norms: nsq[1, i] = sum_f hT[f, i]^2
    sq = epool.tile([128, BLOC], mybir.dt.float32, tag="sq")
    nc.scalar.activation(out=sq[:], in_=hT[:],
                         func=mybir.ActivationFunctionType.Square)
    nsq = psum_e.tile([1, BLOC], mybir.dt.float32, tag="nsq")
    for h in range(BLOC // 512):
        nc.tensor.matmul(nsq[:, 512 * h:512 * (h + 1)], ones_col[:],
                         sq[:, 512 * h:512 * (h + 1)], start=True, stop=True)
    # inv = exp(-0.5*ln(nsq) + sb)
    lnn = epool.tile([1, BLOC], mybir.dt.float32, tag="lnn")
    nc.scalar.activation(out=lnn[:], in_=nsq[:], func=mybir.ActivationFunctionType.Ln)
    inv = epool.tile([1, BLOC], mybir.dt.float32, tag="inv")
    nc.scalar.activation(out=inv[:], in_=lnn[:], func=mybir.ActivationFunctionType.Exp,
                         scale=halfneg[:], bias=sb_ap)
    # broadcast inv across partitions via K=1 outer product, then scale
    bc = psum_e.tile([128, BLOC], mybir.dt.float32, tag="bc")
    for h in range(BLOC // 512):
        nc.tensor.matmul(bc[:, 512 * h:512 * (h + 1)], ones_rowf[:, 0:128],
                         inv[:, 512 * h:512 * (h + 1)], start=True, stop=True)
    bc_sb = epool.tile([128, BLOC], mybir.dt.float32, tag="bcsb")
    nc.vector.tensor_copy(bc_sb[:], bc[:])
    nc.vector.tensor_mul(out_sb[:], hT[:], bc_sb[:])


def _build_program(repeat=1, cc=True, big=True):
    nc = bass.Bass()
    dt = mybir.dt
    fs_in = nc.dram_tensor("fs_nat", [BLOC, S], dt.bfloat16, kind="ExternalInput")
    ft_in = nc.dram_tensor("ft_nat", [BLOC, S], dt.bfloat16, kind="ExternalInput")
    wsT_in = nc.dram_tensor("wsT", [S, F], dt.bfloat16, kind="ExternalInput")
    wtT_in = nc.dram_tensor("wtT", [S, F], dt.bfloat16, kind="ExternalInput")
    bs_in = nc.dram_tensor("bs", [1, F], dt.bfloat16, kind="ExternalInput")
    bt_in = nc.dram_tensor("bt", [1, F], dt.bfloat16, kind="ExternalInput")

    fs_out = nc.dram_tensor("fs_out", [128, BLOC], dt.bfloat16, kind="ExternalOutput")
    ft_out = nc.dram_tensor("ft_out", [128, BLOC], dt.bfloat16, kind="ExternalOutput")
    ln_out = nc.dram_tensor("ln_out", [128, 1], dt.float32, kind="ExternalOutput")

    with tile.TileContext(nc) as tc:
        with (
            tc.tile_pool(name="singles", bufs=1) as singles,
            tc.tile_pool(name="pool", bufs=4) as pool,
            tc.tile_pool(name="dram", bufs=1, space="DRAM") as dram,
        ):
            # constants
            ones_row = singles.tile([1, 512], dt.bfloat16)
            nc.vector.memset(ones_row[:], 1.0)
            ones_rowf = singles.tile([1, 128], dt.float32)
            nc.vector.memset(ones_rowf[:], 1.0)
            ones_col = singles.tile([128, 1], dt.float32)
            nc.vector.memset(ones_col[:], 1.0)
            halfneg = singles.tile([1, 1], dt.float32)
            nc.vector.memset(halfneg[:], -0.5)
            lninvT = singles.tile([1, 1], dt.float32)
            nc.vector.memset(lninvT[:], float(-np.log(T)))
            cbias = singles.tile([128, 1], dt.float32)
            nc.vector.memset(cbias[:], C)
            nscale = singles.tile([128, 1], dt.float32)
            nc.vector.memset(nscale[:], -1.0)

            # weights + biases (host pre-transposed; small)
            wsT_sb = singles.tile([128, KCH * F], dt.bfloat16)
            nc.sync.dma_start(out=wsT_sb[:].rearrange("p (k f) -> p k f", k=KCH),
                              in_=wsT_in[:, :].rearrange("(k p) f -> p k f", p=128))
            wtT_sb = singles.tile([128, KCH * F], dt.bfloat16)
            nc.sync.dma_start(out=wtT_sb[:].rearrange("p (k f) -> p k f", k=KCH),
                              in_=wtT_in[:, :].rearrange("(k p) f -> p k f", p=128))
            bs_sb = singles.tile([1, F], dt.bfloat16)
            nc.sync.dma_start(out=bs_sb[:], in_=bs_in[:])
            bt_sb = singles.tile([1, F], dt.bfloat16)
            nc.sync.dma_start(out=bt_sb[:], in_=bt_in[:])

            ftT_loc = singles.tile([128, BLOC], dt.bfloat16)
            fsT_sc = singles.tile([128, BLOC], dt.bfloat16)

            # repeat>1 replicates the whole compute body for slope-based HW
            # timing; the grading path uses repeat=1.
            for rep in range(repeat):
              with (
                  tc.tile_pool(name="psum_e", bufs=1, space="PSUM") as psum_e,
                  tc.tile_pool(name="epool", bufs=1) as epool,
              ):
                  consts_t = (ones_row, ones_rowf, ones_col, halfneg, 0.0)
                  _embed_side(nc, pool, epool, psum_e, consts_t, ft_in, wtT_sb,
                              bt_sb, ftT_loc)
                  consts_s = (ones_row, ones_rowf, ones_col, halfneg, lninvT[:])
                  _embed_side(nc, pool, epool, psum_e, consts_s, fs_in, wsT_sb,
                              bs_sb, fsT_sc)

              # ship embeddings to host
              nc.sync.dma_start(out=fs_out[:], in_=fsT_sc[:])
              nc.sync.dma_start(out=ft_out[:], in_=ftT_loc[:])

              # AllGather ftT blocks -> ftT_full [128, B]
              cc_in = dram.tile([128, BLOC], dt.bfloat16)
              cc_out = nc.dram_tensor(f"cc_out_sh{rep}", [NCORES * 128, BLOC],
                                      dt.bfloat16, addr_space="Shared")
              nc.gpsimd.dma_start(out=cc_in[:], in_=ftT_loc[:])
              if cc:
                  nc.gpsimd.collective_compute(
                      "AllGather", mybir.AluOpType.bypass,
                      replica_groups=[list(range(NCORES))],
                      ins=[cc_in.opt()], outs=[cc_out.ap().opt()],
                  )
              else:  # timing variant: fake the gather with local copies
                  for m in range(NCORES):
                      nc.gpsimd.dma_start(out=cc_out[128 * m:128 * (m + 1), :],
                                          in_=cc_in[:])
              ftT_full = singles.tile([128, B], dt.bfloat16)
              nc.sync.dma_start(
                  out=ftT_full[:].rearrange("p (m r) -> p m r", m=NCORES),
                  in_=cc_out[:, :].rearrange("(m p) r -> p m r", p=128))

              # big sweep: one [1024 x 8192] block of s per core
              products = singles.tile([128, RCH * BLOC], dt.float32)
              if not big:
                  nc.vector.memset(products[:, 0:8], 1.0)
              with (
                  tc.tile_pool(name="bigpsum", bufs=2, space="PSUM") as bigpsum,
                  tc.tile_pool(name="vpool", bufs=2) as vpool,
                  tc.tile_pool(name="lpool", bufs=1) as lpool,
              ):
                  for r in range(RCH if big else 0):
                      lhs = fsT_sc[:, 128 * r:128 * (r + 1)]
                      vrow = vpool.tile([128, B], dt.float32, tag="vrow")
                      for g in range(NGRP):
                          ps = bigpsum.tile([128, GCOLS], dt.float32, tag="grp")
                          for j in range(GCOLS // 512):
                              c0 = GCOLS * g + 512 * j
                              nc.tensor.matmul(ps[:, 512 * j:512 * (j + 1)], lhs,
                                               ftT_full[:, c0:c0 + 512],
                                               start=True, stop=True)
                          nc.scalar.activation(
                              out=vrow[:, GCOLS * g:GCOLS * (g + 1)], in_=ps[:],
                              func=mybir.ActivationFunctionType.Sigmoid,
                              bias=cbias[:], scale=nscale[:])
                      # pairwise-product tree over the whole row chunk
                      v2 = vrow[:].rearrange("p (n two) -> p n two", two=2)
                      l1 = lpool.tile([128, B // 2], dt.float32, tag="l1")
                      nc.vector.tensor_mul(l1[:], v2[:, :, 0], v2[:, :, 1])
                      l1r = l1[:].rearrange("p (n two) -> p n two", two=2)
                      l2 = lpool.tile([128, B // 4], dt.float32, tag="l2")
                      nc.vector.tensor_mul(l2[:], l1r[:, :, 0], l1r[:, :, 1])
                      l2r = l2[:].rearrange("p (n two) -> p n two", two=2)
                      nc.vector.tensor_mul(products[:, BLOC * r:BLOC * (r + 1)],
                                           l2r[:, :, 0], l2r[:, :, 1])

              # final: sum of ln of all chunk products (in-place over products)
              lnacc = singles.tile([128, 1], dt.float32)
              nc.scalar.activation(out=products[:], in_=products[:],
                                   func=mybir.ActivationFunctionType.Ln,
                                   accum_out=lnacc[:])
              nc.sync.dma_start(out=ln_out[:], in_=lnacc[:])

    _fix_multiwaits(nc)
    return nc


_NC_CACHE = {}


def _get_program(repeat=1):
    if repeat not in _NC_CACHE:
        _NC_CACHE[repeat] = _build_program(repeat)
    return _NC_CACHE[repeat]


def _to_bf16(a):
    """fp32 -> bf16 via integer round-to-nearest-even (fast path)."""
    u = np.ascontiguousarray(a, np.float32).view(np.uint32)
    r = ((u + np.uint32(0x7FFF) + ((u >> np.uint32(16)) & np.uint32(1)))
         >> np.uint32(16)).astype(np.uint16)
    return r.view(BF16).reshape(a.shape)


def kernel(f_s, f_t, y, W_s, b_s, W_t, b_t, _return_trace=False, _repeat=1,
           **_ignored):
    f_s = np.asarray(f_s, np.float32)
    f_t = np.asarray(f_t, np.float32)
    W_s = np.asarray(W_s, np.float32)
    W_t = np.asarray(W_t, np.float32)
    b_s = np.asarray(b_s, np.float32)
    b_t = np.asarray(b_t, np.float32)
    y = np.asarray(y)

    fs_bf = _to_bf16(f_s)          # [B, S] natural layout; device transposes
    ft_bf = _to_bf16(f_t)
    wsT = _to_bf16(np.ascontiguousarray(W_s.T))   # [S, F]
    wtT = _to_bf16(np.ascontiguousarray(W_t.T))
    bs_row = _to_bf16(b_s.reshape(1, F))
    bt_row = _to_bf16(b_t.reshape(1, F))

    in_maps = []
    for m in range(NCORES):
        sl = slice(BLOC * m, BLOC * (m + 1))
        in_maps.append({
            "fs_nat": fs_bf[sl], "ft_nat": ft_bf[sl],
            "wsT": wsT, "wtT": wtT, "bs": bs_row, "bt": bt_row,
        })

    nc = _get_program(_repeat)
    res = run_bass_kernel_spmd(nc, in_maps, list(range(NCORES)))

    # host-side assembly (fp64)
    sum_lnv = 0.0
    fss_rows = []
    ft_rows = []
    for m in range(NCORES):
        r = res.results[m]
        sum_lnv += np.asarray(r["ln_out"], np.float64).sum()
        fss_rows.append(np.asarray(r["fs_out"], np.float64).T)  # [BLOC, F], scaled 1/T
        ft_rows.append(np.asarray(r["ft_out"], np.float64).T)
    fss = np.concatenate(fss_rows, axis=0)  # [B, F]
    ft = np.concatenate(ft_rows, axis=0)

    # SUM_ij L_ij = B^2*c - sum_lnv   (ln sigmoid(c-s) = c - L)
    L_big = B * B * C - sum_lnv
    d = np.einsum("if,if->i", fss, ft)            # diagonal s_ii
    L_diag = np.logaddexp(d, C).sum()             # sum_i ln(e^{d_i} + eps)
    n_classes = 100
    yi = y.astype(np.int64)
    Fss = np.zeros((n_classes, F))
    Ftc = np.zeros((n_classes, F))
    np.add.at(Fss, yi, fss)
    np.add.at(Ftc, yi, ft)
    cls = np.einsum("cf,cf->", Fss, Ftc)
    n_c = np.bincount(yi, minlength=n_classes).astype(np.float64)
    n_neg = B * B - np.sum(n_c ** 2)

    loss = (L_big - L_diag + d.sum() - cls - n_neg * C) / B
    out = np.float32(loss)
    if _return_trace:
        return out, res
    return out
